# revision 11
# baseline (speedup 1.0000x reference)
"""MoE layer (8 experts, top-2) on 8 Trainium2 NeuronCores.

Expert parallelism with host-side dispatch; fp8 DoubleRow matmuls with full
error compensation, all accumulating in a single fp32 PSUM group per output
tile:
  - Host: gate logits, top-2 + softmax, token->expert dispatch. Gate scales
    are folded into x (relu is positive-homogeneous), so the device computes
    plain y_e = relu(x_e @ w1.T) @ w2.T on pre-scaled tokens.
  - Every operand T is split T = T0 + T1 with both halves fp8 e4m3. The
    residual product rides in the same PSUM at matched scale by pre-scaling
    the weight residual up by 2^4 and the activation main down by 2^4 (both
    exact exponent shifts in fp8):
      psum = x0@w0.T + x1@w0.T + (x0/16)@((w-w0)*16).T
    dropping only the tiny residual*residual term. Each matmul pairs two
    128-deep k-tiles in MatmulPerfMode.DoubleRow (0.5 cycles/row = 4x bf16
    throughput), so the compensated total runs at 1.5x bf16 speed with
    bf16-level accuracy (~3e-3 end to end).
  - Layer 1: h0 = fp8(relu(psum)) (ACT), h1 = fp8(relu(psum)-h0) (DVE stt),
    h0d = h0/16 (ACT). Layer 2 repeats the same 3-group pattern on
    (h0, h1, h0d) against w2 splits, yT written back d-major.
  - Both layers keep tokens on the moving free dim: any block width, no
    128-token padding anywhere (capacity = max expert count, exactly).
  - DMA issue order is hand-scheduled: block0 x + first w1 chunks feed the
    first matmuls within ~2.5us; the w1 chunk stream stays ahead of L1; w2a
    then w2b follow so layer 2's mains/corrections are resident just in time.
"""

import os

os.environ.setdefault("BASS_NEVER_TRACE", "1")

import numpy as np
import ml_dtypes

D_MODEL = 1024
D_FF = 4096
NUM_EXPERTS = 8
TOP_K = 2
P = 128
KD = D_MODEL // P  # 8
KF = D_FF // P  # 32
C_BLK = 512
WCH = 4  # fc chunk size for w1 DMA staging
SC = 16.0  # residual pre-scale (2^4); shifted operands use 1/SC

F8 = ml_dtypes.float8_e4m3

_NC_CACHE: dict[int, object] = {}


def _block_widths(C: int) -> list[int]:
    """First block 512 (buys time for the w2 DMA stream before L2 starts),
    the rest equal-ish. Every block should be >=342 wide so the PE engine
    time per DoubleRow (0.5 cyc/row) stays above the 71 ns sequencer cost."""
    if C <= C_BLK:
        return [C]
    nb = -(-C // C_BLK)
    rest = C - C_BLK
    base = rest // (nb - 1)
    remn = rest % (nb - 1)
    widths = [C_BLK] + [base + (1 if i < remn else 0) for i in range(nb - 1)]
    assert sum(widths) == C
    return widths


def capacity(max_count: int) -> int:
    return max(max_count, 2 * 342)


def build_moe_nc(C: int):
    """Bass/Tile program for one expert shard with token capacity C.

    DRAM inputs (per core), all fp8 e4m3:
      xs0 [128, KD, C]        xs0[p,k,c] = fp8(g_c * x_c)[k*128+p]
      xs1 [128, KD, C]        fp8 residual of the above
      xsd [128, KD, C]        fp8(xs0 / 16) (exact shift)
      w1a [128, KF, KD, 128]  w1a[p,fc,k,j] = fp8(w1[fc*128+j, k*128+p])
      w1b [128, KF, KD, 128]  fp8((w1 - w1a)*16), same layout
      w2a [128, KF, D]        w2a[p,kf,d] = fp8(w2[d, kf*128+p])
      w2b [128, KF, D]        fp8((w2 - w2a)*16)
    DRAM output:
      yT  [128, KD, C] f32    yT[p,dt,c] = y[c, dt*128+p]
    """
    import concourse.mybir as mybir
    import concourse.tile as tile
    from concourse import bacc

    f8, f32 = mybir.dt.float8e4, mybir.dt.float32
    Relu = mybir.ActivationFunctionType.Relu
    DR = mybir.MatmulPerfMode.DoubleRow
    Alu = mybir.AluOpType

    widths = _block_widths(C)
    NB = len(widths)

    nc = bacc.Bacc("TRN2", target_bir_lowering=False, debug=False)
    # x inputs are laid out per 512-padded block so every DMA descriptor is a
    # full 512B contiguous run (keeps the DMA at full rate)
    xs0 = nc.dram_tensor("xs0", [P, KD, NB, C_BLK], f8, kind="ExternalInput")
    xs1 = nc.dram_tensor("xs1", [P, KD, NB, C_BLK], f8, kind="ExternalInput")
    xsd = nc.dram_tensor("xsd", [P, KD, NB, C_BLK], f8, kind="ExternalInput")
    w1a = nc.dram_tensor("w1a", [P, KF, KD, P], f8, kind="ExternalInput")
    w1b = nc.dram_tensor("w1b", [P, KF, KD, P], f8, kind="ExternalInput")
    w2a = nc.dram_tensor("w2a", [P, KF, D_MODEL], f8, kind="ExternalInput")
    w2b = nc.dram_tensor("w2b", [P, KF, D_MODEL], f8, kind="ExternalInput")
    yT = nc.dram_tensor("yT", [P, KD, C], f32, kind="ExternalOutput")

    blocks = []
    off = 0
    for w in widths:
        blocks.append((off, w))
        off += w

    with tile.TileContext(nc) as tc:
        with (
            tc.tile_pool(name="wpool", bufs=1) as wpool,
            tc.tile_pool(name="xpool", bufs=2) as xpool,
            tc.tile_pool(name="hpool", bufs=1) as hpool,
            tc.tile_pool(name="ypool", bufs=3) as ypool,
            tc.tile_pool(name="pmp", bufs=4, space="PSUM") as pmp,
            tc.tile_pool(name="pymp", bufs=3, space="PSUM") as pymp,
        ):
            # ---- SBUF weight tiles (resident) ----
            w1a_ch = [
                wpool.tile([P, WCH, KD, P], f8, tag=f"w1a_{c0}", name=f"w1a_{c0}")
                for c0 in range(0, KF, WCH)
            ]
            w1b_ch = [
                wpool.tile([P, WCH, KD, P], f8, tag=f"w1b_{c0}", name=f"w1b_{c0}")
                for c0 in range(0, KF, WCH)
            ]
            w2a_t = wpool.tile([P, KF, D_MODEL], f8, tag="w2a", name="w2a_t")
            w2b_t = wpool.tile([P, KF, D_MODEL], f8, tag="w2b", name="w2b_t")

            # ---- block 0 x tiles + head-of-stream DMAs (criticality order:
            # each piece lands just before the matmul group that reads it) ----
            xt0 = xpool.tile([P, KD, C_BLK], f8, tag="xt0", name="xt0_0")
            xt1 = xpool.tile([P, KD, C_BLK], f8, tag="xt1", name="xt1_0")
            xtd = xpool.tile([P, KD, C_BLK], f8, tag="xtd", name="xtd_0")
            nc.sync.dma_start(w1a_ch[0][:, 0:1], w1a[:, 0:1])
            for k0 in range(0, KD, 2):
                nc.sync.dma_start(xt0[:, k0 : k0 + 2], xs0[:, k0 : k0 + 2, 0])
            for k0 in range(0, KD, 2):
                nc.sync.dma_start(xt1[:, k0 : k0 + 2], xs1[:, k0 : k0 + 2, 0])
            for k0 in range(0, KD, 2):
                nc.sync.dma_start(xtd[:, k0 : k0 + 2], xsd[:, k0 : k0 + 2, 0])
            nc.sync.dma_start(w1b_ch[0][:, 0:1], w1b[:, 0:1])
            for j in range(1, WCH):
                nc.sync.dma_start(w1a_ch[0][:, j : j + 1], w1a[:, j : j + 1])
                nc.sync.dma_start(w1b_ch[0][:, j : j + 1], w1b[:, j : j + 1])
            # w1 chunk stream (stays well ahead of L1 consumption), then w2a
            # halves (layer-2 mains), then w2b halves (layer-2 corrections)
            for c0 in range(WCH, KF, WCH):
                ci = c0 // WCH
                nc.sync.dma_start(w1a_ch[ci][:], w1a[:, c0 : c0 + WCH])
                nc.sync.dma_start(w1b_ch[ci][:], w1b[:, c0 : c0 + WCH])
            DH = D_MODEL // 2
            for d0 in (0, DH):
                nc.sync.dma_start(w2a_t[:, :, d0 : d0 + DH], w2a[:, :, d0 : d0 + DH])
            for d0 in (0, DH):
                nc.sync.dma_start(w2b_t[:, :, d0 : d0 + DH], w2b[:, :, d0 : d0 + DH])

            def w1a_ap(fc, kp):
                return w1a_ch[fc // WCH][:, fc % WCH, 2 * kp : 2 * kp + 2, :]

            def w1b_ap(fc, kp):
                return w1b_ch[fc // WCH][:, fc % WCH, 2 * kp : 2 * kp + 2, :]

            xts = {0: (xt0, xt1, xtd)}
            for bi, (off, w) in enumerate(blocks):
                xt0, xt1, xtd = xts.pop(bi)
                h0 = hpool.tile([P, KF, C_BLK], f8, tag="h0", name=f"h0_{bi}")
                h1 = hpool.tile([P, KF, C_BLK], f8, tag="h1", name=f"h1_{bi}")
                h0d = hpool.tile([P, KF, C_BLK], f8, tag="h0d", name=f"h0d_{bi}")
                # ---- layer 1: 12 DoubleRows into one PSUM per f-tile ----
                for fc in range(KF):
                    pm = pmp.tile([P, C_BLK], f32, tag="pm", name=f"pm_{bi}_{fc}")
                    for kp in range(KD // 2):
                        nc.tensor.matmul(
                            pm[:, :w], lhsT=w1a_ap(fc, kp),
                            rhs=xt0[:, 2 * kp : 2 * kp + 2, :w],
                            start=(kp == 0), stop=False, perf_mode=DR,
                        )
                    for kp in range(KD // 2):
                        nc.tensor.matmul(
                            pm[:, :w], lhsT=w1a_ap(fc, kp),
                            rhs=xt1[:, 2 * kp : 2 * kp + 2, :w],
                            start=False, stop=False, perf_mode=DR,
                        )
                    for kp in range(KD // 2):
                        nc.tensor.matmul(
                            pm[:, :w], lhsT=w1b_ap(fc, kp),
                            rhs=xtd[:, 2 * kp : 2 * kp + 2, :w],
                            start=False, stop=(kp == KD // 2 - 1), perf_mode=DR,
                        )
                    nc.scalar.activation(h0[:, fc, :w], pm[:, :w], Relu)
                    nc.vector.scalar_tensor_tensor(
                        h1[:, fc, :w], pm[:, :w], 0.0, h0[:, fc, :w],
                        Alu.max, Alu.subtract,
                    )
                    nc.scalar.mul(h0d[:, fc, :w], h0[:, fc, :w], 1.0 / SC)
                # prefetch next block's x before layer 2's y DMAs hit the queue
                if bi + 1 < len(blocks):
                    nxt0 = xpool.tile([P, KD, C_BLK], f8, tag="xt0", name=f"xt0_{bi+1}")
                    nxt1 = xpool.tile([P, KD, C_BLK], f8, tag="xt1", name=f"xt1_{bi+1}")
                    nxtd = xpool.tile([P, KD, C_BLK], f8, tag="xtd", name=f"xtd_{bi+1}")
                    nc.sync.dma_start(nxt0[:], xs0[:, :, bi + 1])
                    nc.sync.dma_start(nxt1[:], xs1[:, :, bi + 1])
                    nc.sync.dma_start(nxtd[:], xsd[:, :, bi + 1])
                    xts[bi + 1] = (nxt0, nxt1, nxtd)
                # ---- layer 2: 48 DoubleRows into one PSUM per d-tile ----
                for dt in range(KD):
                    pym = pymp.tile([P, C_BLK], f32, tag="pym", name=f"pym_{bi}_{dt}")
                    dsl = slice(dt * P, (dt + 1) * P)
                    for kp in range(KF // 2):
                        nc.tensor.matmul(
                            pym[:, :w], lhsT=w2a_t[:, 2 * kp : 2 * kp + 2, dsl],
                            rhs=h0[:, 2 * kp : 2 * kp + 2, :w],
                            start=(kp == 0), stop=False, perf_mode=DR,
                        )
                    for kp in range(KF // 2):
                        nc.tensor.matmul(
                            pym[:, :w], lhsT=w2a_t[:, 2 * kp : 2 * kp + 2, dsl],
                            rhs=h1[:, 2 * kp : 2 * kp + 2, :w],
                            start=False, stop=False, perf_mode=DR,
                        )
                    for kp in range(KF // 2):
                        nc.tensor.matmul(
                            pym[:, :w], lhsT=w2b_t[:, 2 * kp : 2 * kp + 2, dsl],
                            rhs=h0d[:, 2 * kp : 2 * kp + 2, :w],
                            start=False, stop=(kp == KF // 2 - 1), perf_mode=DR,
                        )
                    yt = ypool.tile([P, C_BLK], f32, tag="yt", name=f"yt_{bi}_{dt}")
                    if bi == len(blocks) - 1 and dt == KD - 1:
                        # drain the tail in slices so copy/DMA/sem pipeline
                        ns = 4
                        sl = -(-w // ns)
                        for q0 in range(0, w, sl):
                            q1 = min(q0 + sl, w)
                            nc.scalar.copy(yt[:, q0:q1], pym[:, q0:q1])
                            nc.sync.dma_start(
                                yT[:, dt, off + q0 : off + q1], yt[:, q0:q1]
                            )
                    else:
                        nc.scalar.copy(yt[:, :w], pym[:, :w])
                        nc.sync.dma_start(yT[:, dt, off : off + w], yt[:, :w])

    nc.compile()
    return nc


def route_tokens(xf: np.ndarray, gate_w: np.ndarray):
    """Top-2 routing, replicating jax.lax.top_k tie-breaking (lowest index)."""
    logits = xf @ gate_w.astype(np.float32).T  # [T, E]
    top2 = np.argsort(-logits, axis=-1, kind="stable")[:, :TOP_K]
    tv = np.take_along_axis(logits, top2, axis=-1)
    tv = tv - tv.max(axis=-1, keepdims=True)
    ex = np.exp(tv)
    gates = ex / ex.sum(axis=-1, keepdims=True)
    rows, weights = [], []
    for e in range(NUM_EXPERTS):
        r, kpos = np.nonzero(top2 == e)
        rows.append(r)
        weights.append(gates[r, kpos].astype(np.float32))
    return rows, weights


def _fp8_pair(a: np.ndarray, scale: float = 1.0):
    """a ~= a0 + a1/scale with both halves fp8 e4m3."""
    a0 = a.astype(F8)
    a1 = ((a - a0.astype(np.float32)) * scale).astype(F8)
    return a0, a1


def make_expert_inputs(xf, w1, w2, rows, weights, C):
    """Per-core input arrays in the DRAM layouts build_moe_nc expects."""
    widths = _block_widths(C)
    NB = len(widths)
    starts = np.cumsum([0] + widths[:-1])
    in_maps = []
    for e in range(NUM_EXPERTS):
        cnt = len(rows[e])
        Xg = np.zeros((C, D_MODEL), np.float32)
        Xg[:cnt] = xf[rows[e]] * weights[e][:, None]
        X0, X1 = _fp8_pair(Xg)
        Xd = (X0.astype(np.float32) / SC).astype(F8)

        def xlay(X):
            # [P, KD, C] -> 512-padded blocks [P, KD, NB, 512]
            xt = X.T.reshape(KD, P, C).transpose(1, 0, 2)  # [P, KD, C]
            out = np.zeros((P, KD, NB, C_BLK), X.dtype)
            for bi, (o, w) in enumerate(zip(starts, widths)):
                out[:, :, bi, :w] = xt[:, :, o : o + w]
            return out

        W1_0, W1_1 = _fp8_pair(w1[e].astype(np.float32), SC)

        def w1lay(W):
            return np.ascontiguousarray(W.reshape(KF, P, KD, P).transpose(3, 0, 2, 1))

        W2_0, W2_1 = _fp8_pair(w2[e].astype(np.float32), SC)

        def w2lay(W):
            return np.ascontiguousarray(W.T.reshape(KF, P, D_MODEL).transpose(1, 0, 2))

        in_maps.append(
            {
                "xs0": xlay(X0),
                "xs1": xlay(X1),
                "xsd": xlay(Xd),
                "w1a": w1lay(W1_0),
                "w1b": w1lay(W1_1),
                "w2a": w2lay(W2_0),
                "w2b": w2lay(W2_1),
            }
        )
    return in_maps


def kernel(x, gate_w, w1, w2):
    from concourse.bass_utils import run_bass_kernel_spmd

    x = np.asarray(x)
    gate_w = np.asarray(gate_w)
    w1 = np.asarray(w1)
    w2 = np.asarray(w2)
    B, S, D = x.shape

    xf = x.reshape(-1, D).astype(np.float32)
    rows, weights = route_tokens(xf, gate_w)
    counts = [len(r) for r in rows]
    C = capacity(max(counts))

    nc = _NC_CACHE.get(C)
    if nc is None:
        nc = _NC_CACHE[C] = build_moe_nc(C)
    in_maps = make_expert_inputs(xf, w1, w2, rows, weights, C)
    res = run_bass_kernel_spmd(nc, in_maps, core_ids=list(range(NUM_EXPERTS)))

    out = np.zeros((B * S, D), np.float32)
    for e in range(NUM_EXPERTS):
        yT = res.results[e]["yT"]  # [P, KD, C]
        y = yT.transpose(2, 1, 0).reshape(C, D_MODEL)
        out[rows[e]] += y[: counts[e]]
    return out.reshape(B, S, D)


# revision 17
# speedup vs baseline: 1.0103x; 1.0103x over previous
"""MoE layer (8 experts, top-2) on 8 Trainium2 NeuronCores.

Expert parallelism with host-side dispatch; fp8 DoubleRow matmuls with full
error compensation, all accumulating in a single fp32 PSUM group per output
tile:
  - Host: gate logits, top-2 + softmax, token->expert dispatch. Gate scales
    are folded into x (relu is positive-homogeneous), so the device computes
    plain y_e = relu(x_e @ w1.T) @ w2.T on pre-scaled tokens.
  - Every operand T is split T = T0 + T1 with both halves fp8 e4m3. The
    residual product rides in the same PSUM at matched scale by pre-scaling
    the weight residual up by 2^4 and the activation main down by 2^4 (both
    exact exponent shifts in fp8):
      psum = x0@w0.T + x1@w0.T + (x0/16)@((w-w0)*16).T
    dropping only the tiny residual*residual term. Each matmul pairs two
    128-deep k-tiles in MatmulPerfMode.DoubleRow (0.5 cycles/row = 4x bf16
    throughput), so the compensated total runs at 1.5x bf16 speed with
    bf16-level accuracy (~3e-3 end to end).
  - Layer 1: h0 = fp8(relu(psum)) (ACT), h1 = fp8(relu(psum)-h0) (DVE stt),
    h0d = h0/16 (ACT). Layer 2 repeats the same 3-group pattern on
    (h0, h1, h0d) against w2 splits, yT written back d-major.
  - Both layers keep tokens on the moving free dim: any block width, no
    128-token padding anywhere (capacity = max expert count, exactly).
  - DMA issue order is hand-scheduled: block0 x + first w1 chunks feed the
    first matmuls within ~2.5us; the w1 chunk stream stays ahead of L1; w2a
    then w2b follow so layer 2's mains/corrections are resident just in time.
"""

import os

os.environ.setdefault("BASS_NEVER_TRACE", "1")

import numpy as np
import ml_dtypes

D_MODEL = 1024
D_FF = 4096
NUM_EXPERTS = 8
TOP_K = 2
P = 128
KD = D_MODEL // P  # 8
KF = D_FF // P  # 32
C_BLK = 512
WCH = 4  # fc chunk size for w1 DMA staging
SC = 16.0  # residual pre-scale (2^4); shifted operands use 1/SC

F8 = ml_dtypes.float8_e4m3

_NC_CACHE: dict[int, object] = {}


def _block_widths(C: int) -> list[int]:
    """First block 512 (buys time for the w2 DMA stream before L2 starts),
    the rest equal-ish. Every block should be >=342 wide so the PE engine
    time per DoubleRow (0.5 cyc/row) stays above the 71 ns sequencer cost."""
    if C <= C_BLK:
        return [C]
    nb = -(-C // C_BLK)
    rest = C - C_BLK
    base = rest // (nb - 1)
    remn = rest % (nb - 1)
    widths = [C_BLK] + [base + (1 if i < remn else 0) for i in range(nb - 1)]
    assert sum(widths) == C
    return widths


def capacity(max_count: int) -> int:
    return max(max_count, 2 * 342)


def build_moe_nc(C: int):
    """Bass/Tile program for one expert shard with token capacity C.

    DRAM inputs (per core), all fp8 e4m3:
      xs0 [128, KD, C]        xs0[p,k,c] = fp8(g_c * x_c)[k*128+p]
      xs1 [128, KD, C]        fp8 residual of the above
      xsd [128, KD, C]        fp8(xs0 / 16) (exact shift)
      w1a [128, KF, KD, 128]  w1a[p,fc,k,j] = fp8(w1[fc*128+j, k*128+p])
      w1b [128, KF, KD, 128]  fp8((w1 - w1a)*16), same layout
      w2a [128, KF, D]        w2a[p,kf,d] = fp8(w2[d, kf*128+p])
      w2b [128, KF, D]        fp8((w2 - w2a)*16)
    DRAM output:
      yT  [128, KD, C] f32    yT[p,dt,c] = y[c, dt*128+p]
    """
    import concourse.mybir as mybir
    import concourse.tile as tile
    from concourse import bacc

    f8, f32 = mybir.dt.float8e4, mybir.dt.float32
    Relu = mybir.ActivationFunctionType.Relu
    DR = mybir.MatmulPerfMode.DoubleRow
    Alu = mybir.AluOpType

    widths = _block_widths(C)
    NB = len(widths)

    nc = bacc.Bacc("TRN2", target_bir_lowering=False, debug=False)
    # x inputs are laid out per 512-padded block so every DMA descriptor is a
    # full 512B contiguous run (keeps the DMA at full rate)
    xs0 = nc.dram_tensor("xs0", [P, KD, NB, C_BLK], f8, kind="ExternalInput")
    xs1 = nc.dram_tensor("xs1", [P, KD, NB, C_BLK], f8, kind="ExternalInput")
    w1a = nc.dram_tensor("w1a", [P, KF, KD, P], f8, kind="ExternalInput")
    w1b = nc.dram_tensor("w1b", [P, KF, KD, P], f8, kind="ExternalInput")
    w2a = nc.dram_tensor("w2a", [P, KF, D_MODEL], f8, kind="ExternalInput")
    w2b = nc.dram_tensor("w2b", [P, KF, D_MODEL], f8, kind="ExternalInput")
    yT = nc.dram_tensor("yT", [P, KD, C], f32, kind="ExternalOutput")

    blocks = []
    off = 0
    for w in widths:
        blocks.append((off, w))
        off += w

    with tile.TileContext(nc) as tc:
        with (
            tc.tile_pool(name="wpool", bufs=1) as wpool,
            tc.tile_pool(name="xpool", bufs=2) as xpool,
            tc.tile_pool(name="hpool", bufs=1) as hpool,
            tc.tile_pool(name="ypool", bufs=3) as ypool,
            tc.tile_pool(name="pmp", bufs=4, space="PSUM") as pmp,
            tc.tile_pool(name="pymp", bufs=3, space="PSUM") as pymp,
        ):
            # ---- SBUF weight tiles (resident) ----
            w1a_ch = [
                wpool.tile([P, WCH, KD, P], f8, tag=f"w1a_{c0}", name=f"w1a_{c0}")
                for c0 in range(0, KF, WCH)
            ]
            w1b_ch = [
                wpool.tile([P, WCH, KD, P], f8, tag=f"w1b_{c0}", name=f"w1b_{c0}")
                for c0 in range(0, KF, WCH)
            ]
            w2a_t = wpool.tile([P, KF, D_MODEL], f8, tag="w2a", name="w2a_t")
            w2b_t = wpool.tile([P, KF, D_MODEL], f8, tag="w2b", name="w2b_t")

            # ---- block 0 x tiles + head-of-stream DMAs (criticality order:
            # each piece lands just before the matmul group that reads it) ----
            xt0 = xpool.tile([P, KD, C_BLK], f8, tag="xt0", name="xt0_0")
            xt1 = xpool.tile([P, KD, C_BLK], f8, tag="xt1", name="xt1_0")
            xtd = xpool.tile([P, KD, C_BLK], f8, tag="xtd", name="xtd_0")
            nc.sync.dma_start(w1a_ch[0][:, 0:1], w1a[:, 0:1])
            for k0 in range(0, KD, 2):
                nc.sync.dma_start(xt0[:, k0 : k0 + 2], xs0[:, k0 : k0 + 2, 0])
            for k0 in range(0, KD, 2):
                nc.sync.dma_start(xt1[:, k0 : k0 + 2], xs1[:, k0 : k0 + 2, 0])
            for k0 in range(0, KD, 2):
                nc.vector.tensor_scalar_mul(
                    xtd[:, k0 : k0 + 2], xt0[:, k0 : k0 + 2], 1.0 / SC
                )
            nc.sync.dma_start(w1b_ch[0][:, 0:1], w1b[:, 0:1])
            for j in range(1, WCH):
                nc.sync.dma_start(w1a_ch[0][:, j : j + 1], w1a[:, j : j + 1])
                nc.sync.dma_start(w1b_ch[0][:, j : j + 1], w1b[:, j : j + 1])
            # w1 chunk stream (stays well ahead of L1 consumption), then w2a
            # halves (layer-2 mains), then w2b halves (layer-2 corrections)
            for c0 in range(WCH, KF, WCH):
                ci = c0 // WCH
                nc.sync.dma_start(w1a_ch[ci][:], w1a[:, c0 : c0 + WCH])
                nc.sync.dma_start(w1b_ch[ci][:], w1b[:, c0 : c0 + WCH])
            DH = D_MODEL // 2
            for d0 in (0, DH):
                nc.sync.dma_start(w2a_t[:, :, d0 : d0 + DH], w2a[:, :, d0 : d0 + DH])
            for d0 in (0, DH):
                nc.sync.dma_start(w2b_t[:, :, d0 : d0 + DH], w2b[:, :, d0 : d0 + DH])

            def w1a_ap(fc, kp):
                return w1a_ch[fc // WCH][:, fc % WCH, 2 * kp : 2 * kp + 2, :]

            def w1b_ap(fc, kp):
                return w1b_ch[fc // WCH][:, fc % WCH, 2 * kp : 2 * kp + 2, :]

            xts = {0: (xt0, xt1, xtd)}
            for bi, (off, w) in enumerate(blocks):
                xt0, xt1, xtd = xts.pop(bi)
                h0 = hpool.tile([P, KF, C_BLK], f8, tag="h0", name=f"h0_{bi}")
                h1 = hpool.tile([P, KF, C_BLK], f8, tag="h1", name=f"h1_{bi}")
                h0d = hpool.tile([P, KF, C_BLK], f8, tag="h0d", name=f"h0d_{bi}")
                # ---- layer 1: 12 DoubleRows into one PSUM per f-tile ----
                for fc in range(KF):
                    pm = pmp.tile([P, C_BLK], f32, tag="pm", name=f"pm_{bi}_{fc}")
                    for kp in range(KD // 2):
                        nc.tensor.matmul(
                            pm[:, :w], lhsT=w1a_ap(fc, kp),
                            rhs=xt0[:, 2 * kp : 2 * kp + 2, :w],
                            start=(kp == 0), stop=False, perf_mode=DR,
                        )
                    for kp in range(KD // 2):
                        nc.tensor.matmul(
                            pm[:, :w], lhsT=w1a_ap(fc, kp),
                            rhs=xt1[:, 2 * kp : 2 * kp + 2, :w],
                            start=False, stop=False, perf_mode=DR,
                        )
                    for kp in range(KD // 2):
                        nc.tensor.matmul(
                            pm[:, :w], lhsT=w1b_ap(fc, kp),
                            rhs=xtd[:, 2 * kp : 2 * kp + 2, :w],
                            start=False, stop=(kp == KD // 2 - 1), perf_mode=DR,
                        )
                    nc.scalar.activation(h0[:, fc, :w], pm[:, :w], Relu)
                    nc.vector.scalar_tensor_tensor(
                        h1[:, fc, :w], pm[:, :w], 0.0, h0[:, fc, :w],
                        Alu.max, Alu.subtract,
                    )
                    nc.scalar.mul(h0d[:, fc, :w], h0[:, fc, :w], 1.0 / SC)
                # prefetch next block's x before layer 2's y DMAs hit the queue
                if bi + 1 < len(blocks):
                    nxt0 = xpool.tile([P, KD, C_BLK], f8, tag="xt0", name=f"xt0_{bi+1}")
                    nxt1 = xpool.tile([P, KD, C_BLK], f8, tag="xt1", name=f"xt1_{bi+1}")
                    nxtd = xpool.tile([P, KD, C_BLK], f8, tag="xtd", name=f"xtd_{bi+1}")
                    nc.sync.dma_start(nxt0[:], xs0[:, :, bi + 1])
                    nc.sync.dma_start(nxt1[:], xs1[:, :, bi + 1])
                    for k0 in range(0, KD, 2):
                        nc.vector.tensor_scalar_mul(
                            nxtd[:, k0 : k0 + 2], nxt0[:, k0 : k0 + 2], 1.0 / SC
                        )
                    xts[bi + 1] = (nxt0, nxt1, nxtd)
                # ---- layer 2: 48 DoubleRows into one PSUM per d-tile ----
                for dt in range(KD):
                    pym = pymp.tile([P, C_BLK], f32, tag="pym", name=f"pym_{bi}_{dt}")
                    dsl = slice(dt * P, (dt + 1) * P)
                    for kp in range(KF // 2):
                        nc.tensor.matmul(
                            pym[:, :w], lhsT=w2a_t[:, 2 * kp : 2 * kp + 2, dsl],
                            rhs=h0[:, 2 * kp : 2 * kp + 2, :w],
                            start=(kp == 0), stop=False, perf_mode=DR,
                        )
                    for kp in range(KF // 2):
                        nc.tensor.matmul(
                            pym[:, :w], lhsT=w2a_t[:, 2 * kp : 2 * kp + 2, dsl],
                            rhs=h1[:, 2 * kp : 2 * kp + 2, :w],
                            start=False, stop=False, perf_mode=DR,
                        )
                    for kp in range(KF // 2):
                        nc.tensor.matmul(
                            pym[:, :w], lhsT=w2b_t[:, 2 * kp : 2 * kp + 2, dsl],
                            rhs=h0d[:, 2 * kp : 2 * kp + 2, :w],
                            start=False, stop=(kp == KF // 2 - 1), perf_mode=DR,
                        )
                    yt = ypool.tile([P, C_BLK], f32, tag="yt", name=f"yt_{bi}_{dt}")
                    nc.scalar.copy(yt[:, :w], pym[:, :w])
                    if bi == len(blocks) - 1 and dt == KD - 1:
                        # two half DMAs on separate queues shorten the drain
                        h = w // 2
                        nc.sync.dma_start(yT[:, dt, off : off + h], yt[:, :h])
                        nc.sync.dma_start(yT[:, dt, off + h : off + w], yt[:, h:w])
                    else:
                        nc.sync.dma_start(yT[:, dt, off : off + w], yt[:, :w])

    nc.compile()
    return nc


def route_tokens(xf: np.ndarray, gate_w: np.ndarray):
    """Top-2 routing, replicating jax.lax.top_k tie-breaking (lowest index)."""
    logits = xf @ gate_w.astype(np.float32).T  # [T, E]
    top2 = np.argsort(-logits, axis=-1, kind="stable")[:, :TOP_K]
    tv = np.take_along_axis(logits, top2, axis=-1)
    tv = tv - tv.max(axis=-1, keepdims=True)
    ex = np.exp(tv)
    gates = ex / ex.sum(axis=-1, keepdims=True)
    rows, weights = [], []
    for e in range(NUM_EXPERTS):
        r, kpos = np.nonzero(top2 == e)
        rows.append(r)
        weights.append(gates[r, kpos].astype(np.float32))
    return rows, weights


def _fp8_pair(a: np.ndarray, scale: float = 1.0):
    """a ~= a0 + a1/scale with both halves fp8 e4m3."""
    a0 = a.astype(F8)
    a1 = ((a - a0.astype(np.float32)) * scale).astype(F8)
    return a0, a1


def make_expert_inputs(xf, w1, w2, rows, weights, C):
    """Per-core input arrays in the DRAM layouts build_moe_nc expects."""
    widths = _block_widths(C)
    NB = len(widths)
    starts = np.cumsum([0] + widths[:-1])
    in_maps = []
    for e in range(NUM_EXPERTS):
        cnt = len(rows[e])
        Xg = np.zeros((C, D_MODEL), np.float32)
        Xg[:cnt] = xf[rows[e]] * weights[e][:, None]
        X0, X1 = _fp8_pair(Xg)

        def xlay(X):
            # [P, KD, C] -> 512-padded blocks [P, KD, NB, 512]
            xt = X.T.reshape(KD, P, C).transpose(1, 0, 2)  # [P, KD, C]
            out = np.zeros((P, KD, NB, C_BLK), X.dtype)
            for bi, (o, w) in enumerate(zip(starts, widths)):
                out[:, :, bi, :w] = xt[:, :, o : o + w]
            return out

        W1_0, W1_1 = _fp8_pair(w1[e].astype(np.float32), SC)

        def w1lay(W):
            return np.ascontiguousarray(W.reshape(KF, P, KD, P).transpose(3, 0, 2, 1))

        W2_0, W2_1 = _fp8_pair(w2[e].astype(np.float32), SC)

        def w2lay(W):
            return np.ascontiguousarray(W.T.reshape(KF, P, D_MODEL).transpose(1, 0, 2))

        in_maps.append(
            {
                "xs0": xlay(X0),
                "xs1": xlay(X1),
                "w1a": w1lay(W1_0),
                "w1b": w1lay(W1_1),
                "w2a": w2lay(W2_0),
                "w2b": w2lay(W2_1),
            }
        )
    return in_maps


def kernel(x, gate_w, w1, w2):
    from concourse.bass_utils import run_bass_kernel_spmd

    x = np.asarray(x)
    gate_w = np.asarray(gate_w)
    w1 = np.asarray(w1)
    w2 = np.asarray(w2)
    B, S, D = x.shape

    xf = x.reshape(-1, D).astype(np.float32)
    rows, weights = route_tokens(xf, gate_w)
    counts = [len(r) for r in rows]
    C = capacity(max(counts))

    nc = _NC_CACHE.get(C)
    if nc is None:
        nc = _NC_CACHE[C] = build_moe_nc(C)
    in_maps = make_expert_inputs(xf, w1, w2, rows, weights, C)
    res = run_bass_kernel_spmd(nc, in_maps, core_ids=list(range(NUM_EXPERTS)))

    out = np.zeros((B * S, D), np.float32)
    for e in range(NUM_EXPERTS):
        yT = res.results[e]["yT"]  # [P, KD, C]
        y = yT.transpose(2, 1, 0).reshape(C, D_MODEL)
        out[rows[e]] += y[: counts[e]]
    return out.reshape(B, S, D)
